# revision 49
# baseline (speedup 1.0000x reference)
"""Trainium2 Bass kernel for masked multi-head attention.

Problem: B=4, N=2048, D=1024, H=16 heads (DK=64).
  q = x @ Wq.T + bq ; k = x @ Wk.T + bk ; v = x @ Wv.T + bv
  scores = q k^T / sqrt(DK), masked (mask==0 -> -1e9), softmax, z = w v

Sharding: 8 cores = 4 batches x 2 head-groups (8 heads each). Each core
gets its batch's x (transposed), its head-group's weight slices
(transposed) and the batch mask (transposed, cast to bf16). Each core
computes z-numerators and softmax denominators for its 8 heads; the host
performs the final divide and concats the results.

Device-side layout (all "transposed", i.e. feature/key dim on partitions):
  Q^T, K^T : [512, 2048]  (head-dim on partitions, 64 per head)
  V'       : per key-chunk [128, 8 heads, 65] = V columns + ones column
  S^T      : [128 keys, 2 heads, 512 queries] in PSUM -- the two heads of a
             pair computed by row-tiled K=64 matmuls (tile T0 rows 0-63,
             T8 rows 64-127) running concurrently.
  Wq (and bq) are pre-scaled by 1/(8*sqrt(512)) on the host so the PSUM
  scores are y/sqrt(512) where y = qk/sqrt(DK).
  P        : exp path split across two engines to break the ACT bottleneck:
             - ACT iters: P = Exp(sqrt(512) * S) on the scalar engine,
               then PM = P * maskT on DVE (bf16 2x mode, one 1024-col inst)
             - DVE iters: PM = (S^2 + B*S + A)^16 * maskT in ONE fused
               custom DVE instruction (quadratic fit of exp(y/16), then
               four squarings; max rel err ~5e-3 over the score range)
  Z'^T[65, 512] += V'[128,65].T @ PM  (row 64 = softmax denominators)
  Za/Zb PSUM accumulators DMA straight to DRAM; the host divides.
Z-matmuls are emitted one iteration behind the S-matmuls so the PE's
in-order queue never waits on the exp/mask chain.
"""

import os
import sys
from contextlib import ExitStack

import numpy as np

for _p in ("/opt/trn_rl_repo", "/root/.axon_site/_ro/trn_rl_repo"):
    if os.path.isdir(_p) and _p not in sys.path:
        sys.path.append(_p)

import ml_dtypes

import concourse.bass as bass
import concourse.tile as tile
from concourse import bacc, mybir
from concourse.bass import broadcast_tensor_aps
from concourse.bass_utils import run_bass_kernel_spmd

B, N, D, H = 4, 2048, 1024, 16
DK = D // H          # 64
HPC = 8              # heads per core
DC = HPC * DK        # 512, per-core model dim
NCORES = 8
BF16 = ml_dtypes.bfloat16

f32 = mybir.dt.float32
bf16 = mybir.dt.bfloat16
AF = mybir.ActivationFunctionType

# exp(y) ~= (s^2 + PB*s + PA)^16 with s = y/sqrt(512), fit on y in [-3.1, 3.1]
PA = 1.0000338161785296
PB = 1.4210126658351314
SQRT512 = float(np.sqrt(512.0))
QSCALE = 1.0 / (8.0 * SQRT512)  # folded into Wq/bq on the host

# which key-chunk iterations run the fused DVE exp (the rest use ACT exp).
# GPS_SET iterations would hand their mask-multiply to GPSIMD — measured
# slower (gpsimd tensor ops ~3x DVE cost + SBUF port contention), so empty.
DVE_SET = frozenset({3, 7, 11, 15})
GPS_SET = frozenset()


def _make_expq16():
    """Register the fused exp-approx+mask custom DVE op (once per process)."""
    import concourse.dve_ops as dve_ops
    from concourse.dve_ops import (
        CUSTOM_DVE_SPECS,
        OPS,
        _CUSTOM_DVE_ROW_BASE,
        _SUB_OPCODE_FOR_NAME,
        DveOp,
    )
    from concourse.dve_spec import C0, C1, Spec, Src0, Src1, lower, sq
    from concourse.dve_uop import DveOpSpec

    name = "EXPQ16_ATTN"
    if name in _SUB_OPCODE_FOR_NAME:
        for op in OPS:
            if op.name == name:
                return op

    # Horner form (s + B)*s + A keeps the dependence chain at 8 ALU stages
    body = sq(sq(sq(sq((Src0 + C1) * Src0 + C0)))) * Src1

    def _ref(in0, in1, c0, c1, c2):
        s = np.asarray(in0, np.float32)
        in1 = np.reshape(np.asarray(in1, np.float32), s.shape)
        q = (s * s + np.float32(c1) * s + np.float32(c0)).astype(np.float32)
        r = (q * q).astype(np.float32)
        r = (r * r).astype(np.float32)
        r = (r * r).astype(np.float32)
        r = (r * r).astype(np.float32)
        return (r * np.asarray(in1, np.float32)).astype(np.float32)

    spec = Spec(body=body, reference=_ref)
    row = _CUSTOM_DVE_ROW_BASE + len(OPS)
    _SUB_OPCODE_FOR_NAME[name] = row
    shas = {}
    for ver in ("v3", "v4"):
        s_ = DveOpSpec(name=name, opcode=row, uops=lower(spec, ver=ver), rd1_en=True)
        shas[ver] = s_.sha(ver)
    op = DveOp(name, spec, subdim=False, uops_sha=shas)
    OPS.append(op)
    CUSTOM_DVE_SPECS[name] = spec
    return op


EXPQ16 = _make_expq16()


def build_bass():
    nc = bacc.Bacc(None, target_bir_lowering=False)

    # All big inputs arrive pre-shuffled to the SBUF layout (partition dim
    # first) so every load is a straight DMA with fat contiguous
    # per-partition lines instead of 1KB strided descriptors.
    xT = nc.dram_tensor("xT", [128, 8, N], bf16, kind="ExternalInput")
    wq = nc.dram_tensor("wq", [128, 8, DC], bf16, kind="ExternalInput")
    wk = nc.dram_tensor("wk", [128, 8, DC], bf16, kind="ExternalInput")
    wv = nc.dram_tensor("wv", [128, 8, DC], bf16, kind="ExternalInput")
    bq2 = nc.dram_tensor("bq2", [128, 4], f32, kind="ExternalInput")
    bk2 = nc.dram_tensor("bk2", [128, 4], f32, kind="ExternalInput")
    bvrow = nc.dram_tensor("bvrow", [1, DC], bf16, kind="ExternalInput")
    maskT = nc.dram_tensor("maskT", [128, 16, N], bf16, kind="ExternalInput")
    zT = nc.dram_tensor("zT", [HPC, DK + 1, N], f32, kind="ExternalOutput")

    with tile.TileContext(nc) as tc, ExitStack() as ctx:
        persist = ctx.enter_context(tc.tile_pool(name="persist", bufs=1))

        mask_sb = persist.tile([128, 16, N], bf16)
        QT_sb = persist.tile([128, 4, N], bf16)
        KT_sb = persist.tile([128, 4, N], bf16)
        V_sb = persist.tile([128, 16, HPC, DK + 1], bf16)
        bq_sb = persist.tile([128, 4], f32)
        bk_sb = persist.tile([128, 4], f32)
        bv_sb = persist.tile([1, DC], bf16)
        ones_sb = persist.tile([1, 128], bf16)
        xT_sb = persist.tile([128, 8, N], bf16)
        wq_sb = persist.tile([128, 8, DC], bf16)
        wk_sb = persist.tile([128, 8, DC], bf16)
        wv_sb = persist.tile([128, 8, DC], bf16)

        nc.vector.memset(ones_sb, 1.0)
        nc.vector.memset(V_sb[:, :, :, DK : DK + 1], 1.0)
        nc.sync.dma_start(out=wv_sb, in_=wv[:, :, :])
        for xq in range(4):
            nc.sync.dma_start(
                out=xT_sb[:, :, xq * 512 : (xq + 1) * 512],
                in_=xT[:, :, xq * 512 : (xq + 1) * 512],
            )
        nc.sync.dma_start(out=bv_sb, in_=bvrow[:, :])
        nc.sync.dma_start(out=wq_sb, in_=wq[:, :, :])
        nc.sync.dma_start(out=wk_sb, in_=wk[:, :, :])
        nc.sync.dma_start(out=bq_sb, in_=bq2[:, :])
        nc.sync.dma_start(out=bk_sb, in_=bk2[:, :])
        for mq in range(4):
            nc.sync.dma_start(
                out=mask_sb[:, mq * 4 : (mq + 1) * 4, :],
                in_=maskT[:, mq * 4 : (mq + 1) * 4, :],
            )

        def mm_one(out, lhsT, rhs, start, stop):
            nc.tensor.matmul(out, lhsT=lhsT, rhs=rhs, start=start, stop=stop)

        with tc.tile_pool(name="qkvps", bufs=4, space="PSUM") as qkvps:
            # V first (needs only xT + wv loaded) so the PE has work while
            # the mask/Q/K weights are still streaming in and ACT has nothing
            # to do anyway.  V natural: out[n, d] = x^T.T @ Wv^T + bv
            for mch in range(16):
                ps = qkvps.tile([128, 512], f32, tag="ps")
                for k in range(8):
                    mm_one(
                        ps,
                        xT_sb[:, k, mch * 128 : (mch + 1) * 128],
                        wv_sb[:, k, :],
                        start=(k == 0),
                        stop=False,
                    )
                nc.tensor.matmul(
                    ps, lhsT=ones_sb, rhs=bv_sb, start=False, stop=True
                )
                # ACT is idle in the V phase; DVE copies here leak into the
                # attention phase where DVE is near-saturated.
                nc.scalar.copy(
                    V_sb[:, mch, :, 0:DK],
                    ps.rearrange("p (h d) -> p h d", h=HPC),
                )
            # Q^T and K^T: out[d, n] = sum_k W^T[k, d] * x^T[k, n]; bias is
            # folded into the PSUM->SBUF copy on ACT (per-partition bias).
            # d-chunk-major so head pair 0's Q/K finish first and attention
            # can start while the rest of QKV still runs.
            for dch in range(4):
                for w_sb, b_sb, dst in ((wq_sb, bq_sb, QT_sb), (wk_sb, bk_sb, KT_sb)):
                    for nch in range(4):
                        ps = qkvps.tile([128, 512], f32, tag="ps")
                        for k in range(8):
                            mm_one(
                                ps,
                                w_sb[:, k, dch * 128 : (dch + 1) * 128],
                                xT_sb[:, k, nch * 512 : (nch + 1) * 512],
                                start=(k == 0),
                                stop=(k == 7),
                            )
                        nc.scalar.activation(
                            dst[:, dch, nch * 512 : (nch + 1) * 512],
                            ps,
                            AF.Identity,
                            bias=b_sb[:, dch : dch + 1],
                            scale=1.0,
                        )

        # Attention over head pairs; queries in 512-wide quarters.
        with (
            tc.tile_pool(name="spool", bufs=2, space="PSUM") as spool,
            tc.tile_pool(name="zpool", bufs=4, space="PSUM") as zpool,
            tc.tile_pool(name="pp", bufs=3) as pp,
            tc.tile_pool(name="pmp", bufs=5) as pmp,
            tc.tile_pool(name="zsb", bufs=2) as zsb,
        ):
            blocks = [(hp, nq) for hp in range(HPC // 2) for nq in range(4)]
            zaccs = {}
            pending = []        # (block_idx, zmms_fn) carried across blocks
            out_queue = []      # block outputs, flushed a few iters late so
                                # the copies never stall the ACT/DVE queues
            deferred_mul = None # mask-mul held back behind a custom-DVE op

            def emit_out(bi):
                hp, nq = blocks[bi]
                q0 = nq * 512
                Zat, Zab, Zbt, Zbb = zaccs.pop(bi)
                # Merge the key-half partial sums while staging out of PSUM
                # (PSUM cannot DMA directly).
                za_s = zsb.tile([DK + 1, 512], f32, tag="zs")
                zb_s = zsb.tile([DK + 1, 512], f32, tag="zs")
                nc.scalar.copy(za_s, Zat)
                nc.vector.tensor_add(za_s, za_s, Zab)
                nc.scalar.copy(zb_s, Zbt)
                nc.vector.tensor_add(zb_s, zb_s, Zbb)
                nc.sync.dma_start(out=zT[2 * hp, :, q0 : q0 + 512], in_=za_s)
                nc.sync.dma_start(out=zT[2 * hp + 1, :, q0 : q0 + 512], in_=zb_s)

            for bi, (hp, nq) in enumerate(blocks):
                dch = hp
                q0 = nq * 512
                # Four single-block accumulators: (head A/B) x (key top/bot
                # half). Each is written by exactly one PE row-tile, so the
                # whole attention loop runs in 64x128 row-tiled mode with no
                # mode switches and no cross-tile PSUM bank races.
                Zat = zpool.tile([DK + 1, 512], f32, tag="z")
                Zab = zpool.tile([DK + 1, 512], f32, tag="z")
                Zbt = zpool.tile([DK + 1, 512], f32, tag="z")
                Zbb = zpool.tile([DK + 1, 512], f32, tag="z")
                zaccs[bi] = (Zat, Zab, Zbt, Zbb)
                Zacc = (Zat, Zab, Zbt, Zbb)
                for m in range(16):
                    S = spool.tile([128, 2, 512], f32, tag="s")
                    for j in range(2):
                        off = j * DK
                        nc.tensor.matmul(
                            S[:, j, :],
                            lhsT=KT_sb[
                                off : off + DK, dch, m * 128 : (m + 1) * 128
                            ],
                            rhs=QT_sb[off : off + DK, dch, q0 : q0 + 512],
                            start=True,
                            stop=True,
                        )
                    # Z-matmuls lag two iterations behind: the S-pair for
                    # iteration m enters the PE FIFO ahead of Z(m-2), so the
                    # exp->S chain never waits behind queued Z work, and the
                    # PM inputs are always long since ready. At a block
                    # boundary the new block's Z matmuls are held until m>=4
                    # (its PSUM banks are freed by the merge-adds emitted at
                    # m==3); the backlog drains at two pops per iteration.
                    while pending and len(pending) > 2:
                        if pending[0][0] == bi and m < 4:
                            break
                        pbi, fn, last = pending.pop(0)
                        fn()
                        if last:
                            out_queue.append(pbi)
                    if m == 2:
                        while out_queue:
                            emit_out(out_queue.pop(0))
                    PM = pmp.tile([128, 2, 512], bf16, tag="pm")
                    mask1 = mask_sb[:, m : m + 1, q0 : q0 + 512]  # [128,1,512]
                    if m in DVE_SET:
                        # fused poly-exp + mask in one DVE instruction. It
                        # only depends on the S matmuls, so it must sit in
                        # the DVE queue ahead of the previous iteration's
                        # mask-mul (which waits on ACT's exp) — that mul was
                        # deferred below and is emitted right after.
                        _, mb = broadcast_tensor_aps(S[:, :, :], mask1)
                        nc.vector._custom_dve(
                            EXPQ16, out=PM, in0=S[:, :, :], in1=mb,
                            s0=PA, s1=PB,
                        )
                        if deferred_mul is not None:
                            deferred_mul()
                            deferred_mul = None
                    else:
                        P = pp.tile([128, 2, 512], bf16, tag="p")
                        nc.scalar.activation(P, S, AF.Exp, scale=SQRT512)

                        if m in GPS_SET:
                            for j in range(2):
                                nc.gpsimd.tensor_mul(
                                    PM[:, j, :], P[:, j, :],
                                    mask_sb[:, m, q0 : q0 + 512],
                                )
                        else:
                            def mkmul(P=P, PM=PM, mask1=mask1):
                                _, mb = broadcast_tensor_aps(P[:, :, :], mask1)
                                nc.vector.tensor_mul(PM, P[:, :, :], mb)

                            if (m + 1) in DVE_SET:
                                deferred_mul = mkmul
                            else:
                                mkmul()

                    def zmms(m=m, PM=PM, Zacc=Zacc, hp=hp):
                        Zat, Zab, Zbt, Zbb = Zacc
                        # K=64 row-tiled Z matmuls: T0 (SBUF rows 0-63)
                        # handles the top key-half, T8 the bottom half; the
                        # row group auto-derives from the lhsT partition
                        # base. Interleave T0/T8 jobs so both tiles stream
                        # concurrently.
                        jobs = [
                            (Zat, 0, 0),  # (acc, head j, key half)
                            (Zbb, 1, 1),
                            (Zbt, 1, 0),
                            (Zab, 0, 1),
                        ]
                        for acc, j, half in jobs:
                            p0 = half * 64
                            nc.tensor.matmul(
                                acc,
                                lhsT=V_sb[p0 : p0 + 64, m, 2 * hp + j, :],
                                rhs=PM[p0 : p0 + 64, j, :],
                                start=(m == 0),
                                stop=(m == 15),
                            )

                    pending.append((bi, zmms, m == 15))
            for pbi, fn, last in pending:
                fn()
                if last:
                    out_queue.append(pbi)
            while out_queue:
                emit_out(out_queue.pop(0))

    return nc


def host_prep(x, x_mask, direction, Wq, bq, Wk, bk, Wv, bv):
    """Shard + lay out inputs for the 8 cores. Core c: batch c%4, head-group c//4."""
    x = np.asarray(x, dtype=np.float32)
    x_mask = np.asarray(x_mask)
    direction = int(np.asarray(direction))
    in_maps = []
    for c in range(NCORES):
        b, g = c % 4, c // 4
        rows = slice(g * DC, (g + 1) * DC)
        m = x_mask[b]
        if direction != 0:
            m = m.T
        def shuf(a, groups):
            """[G*128, M] -> [128, G, M] (partition dim first, contiguous)."""
            return np.ascontiguousarray(
                a.reshape(groups, 128, a.shape[-1]).transpose(1, 0, 2)
            )

        in_maps.append(
            {
                "xT": shuf(x[b].T.astype(BF16), 8),
                "wq": shuf(
                    (np.asarray(Wq)[rows].T * np.float32(QSCALE)).astype(BF16), 8
                ),
                "wk": shuf(np.asarray(Wk)[rows].T.astype(BF16), 8),
                "wv": shuf(np.asarray(Wv)[rows].T.astype(BF16), 8),
                "bq2": np.ascontiguousarray(
                    (np.asarray(bq, dtype=np.float32) * np.float32(QSCALE))[rows]
                    .reshape(4, 128)
                    .T
                ),
                "bk2": np.ascontiguousarray(
                    np.asarray(bk, dtype=np.float32)[rows].reshape(4, 128).T
                ),
                "bvrow": np.asarray(bv, dtype=np.float32)[rows]
                .reshape(1, DC)
                .astype(BF16),
                "maskT": shuf(np.asarray(m).astype(BF16), 16),
            }
        )
    return in_maps


def assemble(results):
    """results: per-core dict with 'zT' [8, 65, 2048] -> full z [B, N, D].

    Row 64 of each head's block is the softmax denominator; the divide
    happens here in fp32."""
    z = np.empty((B, N, D), dtype=np.float32)
    for c in range(NCORES):
        b, g = c % 4, c // 4
        zt = np.asarray(results[c]["zT"], dtype=np.float32)  # [8, 65, N]
        zn = zt[:, :DK, :] / zt[:, DK : DK + 1, :]           # [8, 64, N]
        z[b, :, g * DC : (g + 1) * DC] = zn.transpose(2, 0, 1).reshape(N, DC)
    return z


def _ensure_device_backend():
    """Make sure jax's default backend exposes the 8 NeuronCores (the host
    may have flipped jax_platforms to cpu to run the reference)."""
    import jax

    try:
        devs = jax.devices()
    except Exception:
        devs = []
    if len([d for d in devs if d.platform != "cpu"]) < NCORES:
        jax.config.update("jax_platforms", "axon")


def run(inputs, trace=False, tmpdir=None):
    _ensure_device_backend()
    nc = build_bass()
    nc.finalize()
    in_maps = host_prep(**inputs)
    res = run_bass_kernel_spmd(
        nc,
        in_maps,
        core_ids=list(range(NCORES)),
        trace=trace,
        tmpdir=tmpdir,
    )
    return assemble(res.results), res


def kernel(**inputs) -> np.ndarray:
    out, _ = run(inputs)
    return out
